# revision 6
# baseline (speedup 1.0000x reference)
"""Host-side precompute + Bass/Tile kernel emission for the quantum circuit
expectation problem (see numpy_check.py for the validated math).

Restructured circuit (per batch b), state S as 64x64 complex matrix
(rows i = qubits 0-5, cols j = qubits 6-11):
  init: S = 1/64 (uniform, real)                  [= (HS^dag)^{ox12} |0>]
  for l in 0..3:
      S *= D1_b                                   [Rz(x1)^{ox12} diag]
      S = K2_6 @ S @ K2_6^T                       [(SH)^{ox12}]
      S *= D2_b                                   [Rz(x2)^{ox12} diag]
      if l < 3:
          S = U0_l @ S @ V0_l^T + U1_l @ S @ V1_l^T   [K1*CNOTs*T_l folded]
  E = alpha*(|top|^2-|bot|^2) + 2 Re(beta * <top, bot>)

Device layouts (per core, 32 batches):
  A: [128, 2048] f32; partition p = reim*64 + i; free f = b*64 + j
  B: [128, 2048] f32; partition p' = b2*64 + j; free f' = c*128 + reim*64 + i
     (b = 2c + b2, c in [0,16))
PSUM "plane" tiles are [128, 2048] = [re(c,i) 1024 | im(c,i) 1024].
"""
import numpy as np

N_QUBITS = 12
N_LAYERS = 4
BATCH = 256
N_CORES = 8
BPC = BATCH // N_CORES  # 32

_S2 = np.array([[1, 0], [0, 1j]], dtype=np.complex128)
_H2 = np.array([[1, 1], [1, -1]], dtype=np.complex128) / np.sqrt(2)


def _rx(t):
    c, s = np.cos(t / 2), -1j * np.sin(t / 2)
    return np.array([[c, s], [s, c]], dtype=np.complex128)


def _rz(t):
    e = np.exp(-0.5j * t)
    return np.array([[e, 0], [0, np.conj(e)]], dtype=np.complex128)


def _kron_list(ms):
    out = np.array([[1.0]], dtype=np.complex128)
    for m in ms:
        out = np.kron(out, m)
    return out


def _prefxor_perm(nbits):
    n = 2**nbits
    out = np.zeros(n, dtype=np.int64)
    for idx in range(n):
        acc, o = 0, 0
        for k in range(nbits):
            acc ^= (idx >> (nbits - 1 - k)) & 1
            o = (o << 1) | acc
        out[idx] = o
    return out


def _perm_matrix(perm):
    n = len(perm)
    P = np.zeros((n, n), dtype=np.complex128)
    P[perm, np.arange(n)] = 1.0
    return P


def build_operators(params):
    theta = np.asarray(params, np.float64).reshape(N_QUBITS, N_LAYERS, 3)
    K1_1q = _H2 @ _S2.conj().T
    K2_1q = _S2 @ _H2
    K1_6 = _kron_list([K1_1q] * 6)
    K2_6 = _kron_list([K2_1q] * 6)
    cA = _perm_matrix(_prefxor_perm(6))
    cB = _perm_matrix(_prefxor_perm(6))

    T = np.zeros((N_QUBITS, N_LAYERS, 2, 2), dtype=np.complex128)
    for q in range(N_QUBITS):
        for l in range(N_LAYERS):
            T[q, l] = _rx(theta[q, l, 2]) @ _rz(theta[q, l, 1]) @ _rx(theta[q, l, 0])

    K1e = K1_6.copy()
    K1o = K1_6.copy()
    lsb = np.arange(64) & 1
    K1e[:, lsb == 1] = 0.0
    K1o[:, lsb == 0] = 0.0
    Pi32 = _perm_matrix(np.arange(64) ^ 32)

    layers = []
    for l in range(3):
        TA = _kron_list([T[q, l] for q in range(6)])
        TB = _kron_list([T[q, l] for q in range(6, 12)])
        U0 = K1e @ cA @ TA
        U1 = K1o @ cA @ TA
        W0 = K1_6 @ cB
        V0 = W0 @ TB
        V1 = W0 @ Pi32 @ TB
        layers.append((U0, U1, V0, V1))

    Z = np.diag([1.0, -1.0]).astype(np.complex128)
    O = T[0, 3].conj().T @ Z @ T[0, 3]
    return dict(K2_6=K2_6, layers=layers, Omat=O,
                alpha=float(O[0, 0].real), beta=complex(O[0, 1]))


def _realified_T(L):
    """lhsT [128,128] f32 for realified left-mult in A (W~ = [[Lr,-Li],[Li,Lr]])."""
    W = np.block([[L.real, -L.imag], [L.imag, L.real]])
    return np.ascontiguousarray(W.T).astype(np.float32)


def _jside_Ts(L):
    """(WrT, WiT, WmiT) [128,128] f32 each, for I2 (x) L partition transform in B."""
    Wr = np.kron(np.eye(2), L.real)
    Wi = np.kron(np.eye(2), L.imag)
    return (np.ascontiguousarray(Wr.T).astype(np.float32),
            np.ascontiguousarray(Wi.T).astype(np.float32),
            np.ascontiguousarray(-Wi.T).astype(np.float32))


def build_host_data(x, params):
    """Returns in_maps: list (per core) of dict name -> np.ndarray."""
    ops = build_operators(params)
    x = np.asarray(x, dtype=np.float64)
    x1 = np.arcsin(x)
    x2 = np.arccos(x * x)

    pc6 = np.array([bin(i).count("1") for i in range(64)])
    pc12 = pc6[:, None] + pc6[None, :]  # [i, j]

    # pack all [128, x] constants into ONE dram tensor (one DMA, one queue sem)
    pieces = {}
    pieces["ident"] = np.eye(128, dtype=np.float32)
    pieces["k2l"] = _realified_T(ops["K2_6"])
    k2r = _jside_Ts(ops["K2_6"])
    for k in range(3):
        pieces[f"k2r_{k}"] = k2r[k]
    # observable quadratic form: E = v^T M v with M = realified(O (x) I32)
    pieces["obsw"] = _realified_T(np.kron(ops["Omat"], np.eye(32)))
    for l in range(3):
        U0, U1, V0, V1 = ops["layers"][l]
        pieces[f"u0_{l}"] = _realified_T(U0)
        pieces[f"u1_{l}"] = _realified_T(U1)
        v0 = _jside_Ts(V0)
        v1 = _jside_Ts(V1)
        for k in range(3):
            pieces[f"v0_{l}_{k}"] = v0[k]
            pieces[f"v1_{l}_{k}"] = v1[k]
    shared = {}

    in_maps = []
    for core in range(N_CORES):
        bs = np.arange(core * BPC, (core + 1) * BPC)
        m = dict(shared)
        # d1: B layout [b2*64+j, c*64+i];  d2: A layout [i, b*64+j]
        ph1 = np.exp(-1j * x1[bs][:, None, None] * (6 - pc12[None, :, :]))  # [32,64,64] (b,i,j)
        ph2 = np.exp(-1j * x2[bs][:, None, None] * (6 - pc12[None, :, :]))
        phB = ph1.reshape(16, 2, 64, 64).transpose(1, 3, 0, 2).reshape(128, 1024)
        m["cpack"] = np.concatenate([pieces[nm] for nm in PACK1_ORDER], axis=1)
        m["upack"] = np.concatenate([pieces[nm] for nm in PACK2_ORDER], axis=1)
        m["d1r"] = np.ascontiguousarray(phB.real).astype(np.float32)
        m["d1i"] = np.ascontiguousarray(phB.imag).astype(np.float32)
        phA = ph2.transpose(1, 0, 2).reshape(64, 2048)
        m["d2r"] = np.ascontiguousarray(np.tile(phA.real, (2, 1))).astype(np.float32)
        m["d2i"] = np.ascontiguousarray(np.tile(phA.imag, (2, 1))).astype(np.float32)
        in_maps.append(m)
    return in_maps


PACK1_ORDER = ["ident", "k2l", "k2r_0", "k2r_1", "k2r_2"]
PACK2_ORDER = (["obsw"]
               + [f"{nm}_{l}{sfx}" for l in range(3)
                  for nm, sfx in [("u0", ""), ("u1", "")]
                  + [(f"v{v}", f"_{k}") for v in (0, 1) for k in range(3)]])


def dram_dtypes(use_f32r=True):
    import concourse.mybir as mybir
    fr = mybir.dt.float32r if use_f32r else mybir.dt.float32
    return {"cpack": fr, "upack": fr}


def pack_offsets(order):
    offs = {}
    off = 0
    for nm in order:
        offs[nm] = (off, 128)
        off += 128
    return offs, off


# ---------------------------------------------------------------- bass kernel
def emit(ctx, tc, dram, use_f32r=True):
    """Emit kernel IR. dram: dict name -> bass.AP (inputs + 'out' [1,32] f32 output).

    The per-core work is split into two independent batch halves (c-chunks
    0-7 / 8-15); the two half-pipelines interleave so DVE/PE/ACT overlap.
    Half h owns: A-layout free cols [1024h, 1024h+1024); B-layout free cols
    the same (c in [8h, 8h+8)); flips chunks c in [8h, 8h+8).
    """
    import concourse.mybir as mybir

    nc = tc.nc
    FP = mybir.dt.float32
    FR = mybir.dt.float32r
    FW = FR if use_f32r else FP   # matmul operand dtype (TF32-like when f32r)
    ALU = mybir.AluOpType

    consts = ctx.enter_context(tc.tile_pool(name="consts", bufs=1))
    states = ctx.enter_context(tc.tile_pool(name="states", bufs=4))
    temps = ctx.enter_context(tc.tile_pool(name="temps", bufs=4))
    psums = ctx.enter_context(tc.tile_pool(name="psums", bufs=4, space="PSUM"))
    tpsums = ctx.enter_context(tc.tile_pool(name="tpsums", bufs=2, space="PSUM"))

    offs1, packw1 = pack_offsets(PACK1_ORDER)
    CP = consts.tile([128, packw1], FW, name="CP", uniquify=False)
    nc.sync.dma_start(CP[:, :], dram["cpack"][:, :])
    offs2, packw2 = pack_offsets(PACK2_ORDER)
    UP = consts.tile([128, packw2], FW, name="UP", uniquify=False)
    nc.sync.dma_start(UP[:, :], dram["upack"][:, :])

    def cslice(nm):
        if nm in offs1:
            o, w = offs1[nm]
            return CP[:, o:o + w]
        o, w = offs2[nm]
        return UP[:, o:o + w]

    ident = cslice("ident")
    k2l = cslice("k2l")
    k2r = [cslice(f"k2r_{k}") for k in range(3)]
    u0 = [cslice(f"u0_{l}") for l in range(3)]
    u1 = [cslice(f"u1_{l}") for l in range(3)]
    v0 = [[cslice(f"v0_{l}_{k}") for k in range(3)] for l in range(3)]
    v1 = [[cslice(f"v1_{l}_{k}") for k in range(3)] for l in range(3)]
    obsw = cslice("obsw")

    def loadf(name, shape):
        t = consts.tile(shape, FP, name=f"c_{name}", uniquify=False)
        nc.sync.dma_start(t[:, :], dram[name][:, :])
        return t

    # split the d1 loads so early streams' layer-0 shortcut starts sooner
    d1r = consts.tile([128, 1024], FP, name="c_d1r", uniquify=False)
    d1i = consts.tile([128, 1024], FP, name="c_d1i", uniquify=False)
    for q in range(4):
        qs = slice(256 * q, 256 * q + 256)
        nc.sync.dma_start(d1r[:, qs], dram["d1r"][:, qs])
        nc.sync.dma_start(d1i[:, qs], dram["d1i"][:, qs])
    d2r = loadf("d2r", [128, 2048])
    d2i = loadf("d2i", [128, 2048])
    ones = consts.tile([128, 1], FP, name="ones", uniquify=False)
    nc.vector.memset(ones[:, :], 1.0)


    # stream views (4 independent batch quarters) ------------------------
    NS = 4          # streams
    CQ = 16 // NS   # c-chunks per stream (4)
    FQ = 128 * CQ   # free cols per stream tile (512)

    def bplanes(t):
        """B quarter-tile [128,512] -> (re, im) views [128, CQ, 64]."""
        v = t[:, :].rearrange("p (c r i) -> p r c i", r=2, i=64)
        return v[:, 0], v[:, 1]

    def pplanes(t):
        v = t[:, :].rearrange("p (r c i) -> p r c i", r=2, i=64)
        return v[:, 0], v[:, 1]

    def d1q(t2d, h):
        w = FQ // 2
        return t2d[:, w * h:w * h + w].rearrange("p (c i) -> p c i", i=64)

    def t3(t2d):
        return t2d[:, :].rearrange("p (c i) -> p c i", i=64)

    # ---------------- subroutines (per stream) --------------------------
    def diag_B(src_re, src_im, dst, h, from_psum=False):
        # GpSimd cannot read PSUM; when sources are PSUM all ops go to DVE
        eng2 = nc.vector if from_psum else nc.gpsimd
        dre, dim = bplanes(dst)
        t1 = temps.tile([128, FQ // 2], FP, name="t1", tag="t1")
        t2 = temps.tile([128, FQ // 2], FP, name="t2", tag="t2")
        t4 = temps.tile([128, FQ // 2], FP, name="t4", tag="t4")
        nc.vector.tensor_tensor(t3(t1), src_re, d1q(d1r, h), ALU.mult)
        eng2.tensor_tensor(t3(t2), src_im, d1q(d1i, h), ALU.mult)
        nc.vector.tensor_tensor(dre, t3(t1), t3(t2), ALU.subtract)
        nc.vector.tensor_tensor(t3(t1), src_re, d1q(d1i, h), ALU.mult)
        eng2.tensor_tensor(t3(t4), src_im, d1q(d1r, h), ALU.mult)
        nc.vector.tensor_tensor(dim, t3(t1), t3(t4), ALU.add)

    def jmult(trip, src, dst_psum):
        WrT, WiT, WmiT = trip
        sre, sim = bplanes(src)
        o_re = dst_psum[:, 0:FQ // 2]
        o_im = dst_psum[:, FQ // 2:FQ]
        nc.tensor.matmul(o_re, WrT, sre, start=True, stop=False)
        nc.tensor.matmul(o_re, WmiT, sim, start=False, stop=True)
        nc.tensor.matmul(o_im, WiT, sre, start=True, stop=False)
        nc.tensor.matmul(o_im, WrT, sim, start=False, stop=True)

    def jmult2(trip0, src0, trip1, src1, dst_psum):
        s0re, s0im = bplanes(src0)
        s1re, s1im = bplanes(src1)
        o_re = dst_psum[:, 0:FQ // 2]
        o_im = dst_psum[:, FQ // 2:FQ]
        nc.tensor.matmul(o_re, trip0[0], s0re, start=True, stop=False)
        nc.tensor.matmul(o_re, trip0[2], s0im, start=False, stop=False)
        nc.tensor.matmul(o_re, trip1[0], s1re, start=False, stop=False)
        nc.tensor.matmul(o_re, trip1[2], s1im, start=False, stop=True)
        nc.tensor.matmul(o_im, trip0[1], s0re, start=True, stop=False)
        nc.tensor.matmul(o_im, trip0[0], s0im, start=False, stop=False)
        nc.tensor.matmul(o_im, trip1[1], s1re, start=False, stop=False)
        nc.tensor.matmul(o_im, trip1[0], s1im, start=False, stop=True)

    def imult(lhsT, src, dst_psum):
        nc.tensor.matmul(dst_psum[:, :], lhsT, src[:, :], start=True, stop=True)

    def imult2(lhsT_a, src_a, lhsT_b, src_b, dst_psum):
        nc.tensor.matmul(dst_psum[:, :], lhsT_a, src_a[:, :], start=True, stop=False)
        nc.tensor.matmul(dst_psum[:, :], lhsT_b, src_b[:, :], start=False, stop=True)

    def copyback(dst, src_psum):
        nc.scalar.copy(dst[:, :], src_psum[:, :])

    def flip(src, dst):
        tp = tpsums.tile([128, FQ], FW, name="tp", tag="tp")
        for c in range(CQ):
            nc.tensor.transpose(tp[:, 128 * c:128 * c + 128],
                                src[:, 128 * c:128 * c + 128], ident)
        nc.scalar.copy(dst[:, :], tp[:, :])

    # ---------------- pipeline ------------------------------------------
    cur_planes = [None] * NS
    res = states.tile([1, 32], FP, name="res", tag="res", bufs=1)

    for l in range(4):
        SB2s = [None] * NS
        SB3s = [None] * NS
        SAs = [None] * NS
        pls = [None] * NS
        SA2s = [None] * NS
        SA3s = [None] * NS
        SA4s = [None] * NS
        SBas = [None] * NS
        SBbs = [None] * NS
        for h in range(NS):
            SB2s[h] = states.tile([128, FQ], FW, name=f"SB2_{h}", tag="SB2")
            if l == 0:
                # initial state is uniform 1/64 (real): Dz1 result = tables/64
                dre, dim = bplanes(SB2s[h])
                nc.scalar.mul(dre, d1q(d1r, h), 1.0 / 64.0)
                nc.scalar.mul(dim, d1q(d1i, h), 1.0 / 64.0)
            else:
                diag_B(cur_planes[h][0], cur_planes[h][1], SB2s[h], h,
                       from_psum=True)
        for h in range(NS):
            pk = psums.tile([128, FQ], FP, name=f"pk{h}", tag="pstate")
            jmult(k2r, SB2s[h], pk)
            SB3s[h] = states.tile([128, FQ], FW, name=f"SB3_{h}", tag="SB3")
            sb3re, sb3im = bplanes(SB3s[h])
            pkre, pkim = pplanes(pk)
            nc.scalar.copy(sb3re, pkre)
            nc.scalar.copy(sb3im, pkim)
        for h in range(NS):
            SAs[h] = states.tile([128, FQ], FW, name=f"SA{h}", tag="SA")
            flip(SB3s[h], SAs[h])
        for h in range(NS):
            pls[h] = psums.tile([128, FQ], FP, name=f"pl{h}", tag="pstate")
            imult(k2l, SAs[h], pls[h])
        for h in range(NS):
            pl = pls[h]
            hs = slice(FQ * h, FQ * h + FQ)
            SA2s[h] = states.tile([128, FQ], FW, name=f"SA2_{h}", tag="SA2")
            t1s = temps.tile([128, FQ], FP, name="t1a", tag="t1a")
            t2p = psums.tile([128, FQ], FP, name=f"t2p{h}", tag="t2p", bufs=2)
            nc.vector.tensor_tensor(t1s[:, :], pl[:, :], d2r[:, hs], ALU.mult)
            nc.vector.tensor_tensor(t2p[:, :], pl[:, :], d2i[:, hs], ALU.mult)
            nc.vector.tensor_tensor(SA2s[h][0:64, :], t1s[0:64, :], t2p[64:128, :],
                                    ALU.subtract)
            nc.vector.tensor_tensor(SA2s[h][64:128, :], t1s[64:128, :], t2p[0:64, :],
                                    ALU.add)

        if l == 3:
            for h in range(NS):
                po = psums.tile([128, FQ], FP, name=f"po{h}", tag="pstate")
                imult(obsw, SA2s[h], po)
                PR = states.tile([128, FQ], FP, name=f"PR{h}", tag="SB3")
                sa2f = SA2s[h][:, :].bitcast(FP) if use_f32r else SA2s[h][:, :]
                nc.vector.tensor_tensor(PR[:, :], sa2f, po[:, :], ALU.mult)
                ep = tpsums.tile([1, FQ], FP, name="ep", tag="tp")
                nc.tensor.matmul(ep[:, :], ones[:, :], PR[:, :],
                                 start=True, stop=True)
                epv = ep[:, :].rearrange("p (b j) -> p b j", j=64)
                ro = 8 * h
                nc.vector.tensor_reduce(res[:, ro:ro + 8], epv,
                                        axis=mybir.AxisListType.X, op=ALU.add)
        else:
            for h in range(NS):
                pu = psums.tile([128, FQ], FP, name=f"pu{h}", tag="pstate")
                SA3s[h] = states.tile([128, FQ], FW, name=f"SA3_{h}", tag="SA3")
                imult(u0[l], SA2s[h], pu)
                copyback(SA3s[h], pu)
                pu2 = psums.tile([128, FQ], FP, name=f"pu2{h}", tag="pstate")
                SA4s[h] = states.tile([128, FQ], FW, name=f"SA4_{h}", tag="SA4")
                imult(u1[l], SA2s[h], pu2)
                copyback(SA4s[h], pu2)
            for h in range(NS):
                SBas[h] = states.tile([128, FQ], FW, name=f"SBa{h}", tag="SB3")
                flip(SA3s[h], SBas[h])
                SBbs[h] = states.tile([128, FQ], FW, name=f"SBb{h}", tag="SA")
                flip(SA4s[h], SBbs[h])
            for h in range(NS):
                pv = psums.tile([128, FQ], FP, name=f"pv{h}", tag="pstate")
                jmult2(v0[l], SBas[h], v1[l], SBbs[h], pv)
                # next layer's Dz1 reads this PSUM directly (no copyback)
                cur_planes[h] = pplanes(pv)

    nc.sync.dma_start(dram["out"][:, :], res[:, :])


# ======================================================================
# public entry point
# ======================================================================
_CACHED = {}


def _build_program(use_f32r=True):
    """Build + compile the (input-independent) bass program once."""
    key = bool(use_f32r)
    if key in _CACHED:
        return _CACHED[key]
    from contextlib import ExitStack
    import concourse.bacc as bacc
    import concourse.mybir as mybir
    import concourse.tile as tile

    nc = bacc.Bacc("TRN2", target_bir_lowering=False, debug=False,
                   enable_asserts=True)
    _, packw1 = pack_offsets(PACK1_ORDER)
    _, packw2 = pack_offsets(PACK2_ORDER)
    shapes = {"cpack": [128, packw1], "upack": [128, packw2],
              "d1r": [128, 1024], "d1i": [128, 1024],
              "d2r": [128, 2048], "d2i": [128, 2048]}
    dtmap = dram_dtypes(use_f32r)
    dram = {}
    for name, shape in shapes.items():
        dram[name] = nc.dram_tensor(
            name, shape, dtmap.get(name, mybir.dt.float32),
            kind="ExternalInput").ap()
    dram["out"] = nc.dram_tensor("out", [1, 32], mybir.dt.float32,
                                 kind="ExternalOutput").ap()
    with tile.TileContext(nc) as tc:
        with ExitStack() as ctx:
            emit(ctx, tc, dram, use_f32r=use_f32r)
    nc.compile()
    _CACHED[key] = nc
    return nc


def kernel(x, params):
    """Full-input entry point: x (256,) f32, params (144,) f32 -> (256,) f32.

    Shards the batch over 8 NeuronCores (32 per core), runs the Bass/Tile
    statevector kernel SPMD, gathers per-core expectation values.
    """
    from concourse.bass_utils import run_bass_kernel_spmd

    x = np.asarray(x, dtype=np.float32).reshape(BATCH)
    params = np.asarray(params, dtype=np.float32).reshape(N_QUBITS * N_LAYERS * 3)
    nc = _build_program(use_f32r=True)
    in_maps = build_host_data(x, params)
    res = run_bass_kernel_spmd(nc, in_maps, list(range(N_CORES)))
    out = np.concatenate([res.results[c]["out"].reshape(BPC)
                          for c in range(N_CORES)])
    return out.astype(np.float32)


# revision 7
# speedup vs baseline: 1.0039x; 1.0039x over previous
"""Host-side precompute + Bass/Tile kernel emission for the quantum circuit
expectation problem (see numpy_check.py for the validated math).

Restructured circuit (per batch b), state S as 64x64 complex matrix
(rows i = qubits 0-5, cols j = qubits 6-11):
  init: S = 1/64 (uniform, real)                  [= (HS^dag)^{ox12} |0>]
  for l in 0..3:
      S *= D1_b                                   [Rz(x1)^{ox12} diag]
      S = K2_6 @ S @ K2_6^T                       [(SH)^{ox12}]
      S *= D2_b                                   [Rz(x2)^{ox12} diag]
      if l < 3:
          S = U0_l @ S @ V0_l^T + U1_l @ S @ V1_l^T   [K1*CNOTs*T_l folded]
  E = alpha*(|top|^2-|bot|^2) + 2 Re(beta * <top, bot>)

Device layouts (per core, 32 batches):
  A: [128, 2048] f32; partition p = reim*64 + i; free f = b*64 + j
  B: [128, 2048] f32; partition p' = b2*64 + j; free f' = c*128 + reim*64 + i
     (b = 2c + b2, c in [0,16))
PSUM "plane" tiles are [128, 2048] = [re(c,i) 1024 | im(c,i) 1024].
"""
import numpy as np

N_QUBITS = 12
N_LAYERS = 4
BATCH = 256
N_CORES = 8
BPC = BATCH // N_CORES  # 32

_S2 = np.array([[1, 0], [0, 1j]], dtype=np.complex128)
_H2 = np.array([[1, 1], [1, -1]], dtype=np.complex128) / np.sqrt(2)


def _rx(t):
    c, s = np.cos(t / 2), -1j * np.sin(t / 2)
    return np.array([[c, s], [s, c]], dtype=np.complex128)


def _rz(t):
    e = np.exp(-0.5j * t)
    return np.array([[e, 0], [0, np.conj(e)]], dtype=np.complex128)


def _kron_list(ms):
    out = np.array([[1.0]], dtype=np.complex128)
    for m in ms:
        out = np.kron(out, m)
    return out


def _prefxor_perm(nbits):
    n = 2**nbits
    out = np.zeros(n, dtype=np.int64)
    for idx in range(n):
        acc, o = 0, 0
        for k in range(nbits):
            acc ^= (idx >> (nbits - 1 - k)) & 1
            o = (o << 1) | acc
        out[idx] = o
    return out


def _perm_matrix(perm):
    n = len(perm)
    P = np.zeros((n, n), dtype=np.complex128)
    P[perm, np.arange(n)] = 1.0
    return P


def build_operators(params):
    theta = np.asarray(params, np.float64).reshape(N_QUBITS, N_LAYERS, 3)
    K1_1q = _H2 @ _S2.conj().T
    K2_1q = _S2 @ _H2
    K1_6 = _kron_list([K1_1q] * 6)
    K2_6 = _kron_list([K2_1q] * 6)
    cA = _perm_matrix(_prefxor_perm(6))
    cB = _perm_matrix(_prefxor_perm(6))

    T = np.zeros((N_QUBITS, N_LAYERS, 2, 2), dtype=np.complex128)
    for q in range(N_QUBITS):
        for l in range(N_LAYERS):
            T[q, l] = _rx(theta[q, l, 2]) @ _rz(theta[q, l, 1]) @ _rx(theta[q, l, 0])

    K1e = K1_6.copy()
    K1o = K1_6.copy()
    lsb = np.arange(64) & 1
    K1e[:, lsb == 1] = 0.0
    K1o[:, lsb == 0] = 0.0
    Pi32 = _perm_matrix(np.arange(64) ^ 32)

    layers = []
    for l in range(3):
        TA = _kron_list([T[q, l] for q in range(6)])
        TB = _kron_list([T[q, l] for q in range(6, 12)])
        U0 = K1e @ cA @ TA
        U1 = K1o @ cA @ TA
        W0 = K1_6 @ cB
        V0 = W0 @ TB
        V1 = W0 @ Pi32 @ TB
        layers.append((U0, U1, V0, V1))

    Z = np.diag([1.0, -1.0]).astype(np.complex128)
    O = T[0, 3].conj().T @ Z @ T[0, 3]
    return dict(K2_6=K2_6, layers=layers, Omat=O,
                alpha=float(O[0, 0].real), beta=complex(O[0, 1]))


def _realified_T(L):
    """lhsT [128,128] f32 for realified left-mult in A (W~ = [[Lr,-Li],[Li,Lr]])."""
    W = np.block([[L.real, -L.imag], [L.imag, L.real]])
    return np.ascontiguousarray(W.T).astype(np.float32)


def _jside_Ts(L):
    """(WrT, WiT, WmiT) [128,128] f32 each, for I2 (x) L partition transform in B."""
    Wr = np.kron(np.eye(2), L.real)
    Wi = np.kron(np.eye(2), L.imag)
    return (np.ascontiguousarray(Wr.T).astype(np.float32),
            np.ascontiguousarray(Wi.T).astype(np.float32),
            np.ascontiguousarray(-Wi.T).astype(np.float32))


def build_host_data(x, params):
    """Returns in_maps: list (per core) of dict name -> np.ndarray."""
    ops = build_operators(params)
    x = np.asarray(x, dtype=np.float64)
    x1 = np.arcsin(x)
    x2 = np.arccos(x * x)

    pc6 = np.array([bin(i).count("1") for i in range(64)])
    pc12 = pc6[:, None] + pc6[None, :]  # [i, j]

    # pack all [128, x] constants into ONE dram tensor (one DMA, one queue sem)
    pieces = {}
    pieces["ident"] = np.eye(128, dtype=np.float32)
    pieces["k2l"] = _realified_T(ops["K2_6"])
    k2r = _jside_Ts(ops["K2_6"])
    for k in range(3):
        pieces[f"k2r_{k}"] = k2r[k]
    # observable quadratic form: E = v^T M v with M = realified(O (x) I32)
    pieces["obsw"] = _realified_T(np.kron(ops["Omat"], np.eye(32)))
    for l in range(3):
        U0, U1, V0, V1 = ops["layers"][l]
        pieces[f"u0_{l}"] = _realified_T(U0)
        pieces[f"u1_{l}"] = _realified_T(U1)
        v0 = _jside_Ts(V0)
        v1 = _jside_Ts(V1)
        for k in range(3):
            pieces[f"v0_{l}_{k}"] = v0[k]
            pieces[f"v1_{l}_{k}"] = v1[k]
    shared = {}

    in_maps = []
    for core in range(N_CORES):
        bs = np.arange(core * BPC, (core + 1) * BPC)
        m = dict(shared)
        # d1: B layout [b2*64+j, c*64+i];  d2: A layout [i, b*64+j]
        ph1 = np.exp(-1j * x1[bs][:, None, None] * (6 - pc12[None, :, :]))  # [32,64,64] (b,i,j)
        ph2 = np.exp(-1j * x2[bs][:, None, None] * (6 - pc12[None, :, :]))
        phB = ph1.reshape(16, 2, 64, 64).transpose(1, 3, 0, 2).reshape(128, 1024)
        m["cpack"] = np.concatenate([pieces[nm] for nm in PACK1_ORDER], axis=1)
        m["upack"] = np.concatenate([pieces[nm] for nm in PACK2_ORDER], axis=1)
        m["d1r"] = np.ascontiguousarray(phB.real).astype(np.float32)
        m["d1i"] = np.ascontiguousarray(phB.imag).astype(np.float32)
        phA = ph2.transpose(1, 0, 2).reshape(64, 2048)
        m["d2r"] = np.ascontiguousarray(np.tile(phA.real, (2, 1))).astype(np.float32)
        m["d2i"] = np.ascontiguousarray(np.tile(phA.imag, (2, 1))).astype(np.float32)
        in_maps.append(m)
    return in_maps


PACK1_ORDER = ["ident", "k2l", "k2r_0", "k2r_1", "k2r_2"]
PACK2_ORDER = (["obsw"]
               + [f"{nm}_{l}{sfx}" for l in range(3)
                  for nm, sfx in [("u0", ""), ("u1", "")]
                  + [(f"v{v}", f"_{k}") for v in (0, 1) for k in range(3)]])


def dram_dtypes(use_f32r=True):
    import concourse.mybir as mybir
    fr = mybir.dt.float32r if use_f32r else mybir.dt.float32
    return {"cpack": fr, "upack": fr}


def pack_offsets(order):
    offs = {}
    off = 0
    for nm in order:
        offs[nm] = (off, 128)
        off += 128
    return offs, off


# ---------------------------------------------------------------- bass kernel
def emit(ctx, tc, dram, use_f32r=True):
    """Emit kernel IR. dram: dict name -> bass.AP (inputs + 'out' [1,32] f32 output).

    The per-core work is split into two independent batch halves (c-chunks
    0-7 / 8-15); the two half-pipelines interleave so DVE/PE/ACT overlap.
    Half h owns: A-layout free cols [1024h, 1024h+1024); B-layout free cols
    the same (c in [8h, 8h+8)); flips chunks c in [8h, 8h+8).
    """
    import concourse.mybir as mybir

    nc = tc.nc
    FP = mybir.dt.float32
    FR = mybir.dt.float32r
    FW = FR if use_f32r else FP   # matmul operand dtype (TF32-like when f32r)
    ALU = mybir.AluOpType

    consts = ctx.enter_context(tc.tile_pool(name="consts", bufs=1))
    states = ctx.enter_context(tc.tile_pool(name="states", bufs=4))
    temps = ctx.enter_context(tc.tile_pool(name="temps", bufs=4))
    psums = ctx.enter_context(tc.tile_pool(name="psums", bufs=4, space="PSUM"))
    tpsums = ctx.enter_context(tc.tile_pool(name="tpsums", bufs=2, space="PSUM"))

    offs1, packw1 = pack_offsets(PACK1_ORDER)
    CP = consts.tile([128, packw1], FW, name="CP", uniquify=False)
    nc.sync.dma_start(CP[:, :], dram["cpack"][:, :])
    offs2, packw2 = pack_offsets(PACK2_ORDER)
    UP = consts.tile([128, packw2], FW, name="UP", uniquify=False)
    nc.sync.dma_start(UP[:, :], dram["upack"][:, :])

    def cslice(nm):
        if nm in offs1:
            o, w = offs1[nm]
            return CP[:, o:o + w]
        o, w = offs2[nm]
        return UP[:, o:o + w]

    ident = cslice("ident")
    k2l = cslice("k2l")
    k2r = [cslice(f"k2r_{k}") for k in range(3)]
    u0 = [cslice(f"u0_{l}") for l in range(3)]
    u1 = [cslice(f"u1_{l}") for l in range(3)]
    v0 = [[cslice(f"v0_{l}_{k}") for k in range(3)] for l in range(3)]
    v1 = [[cslice(f"v1_{l}_{k}") for k in range(3)] for l in range(3)]
    obsw = cslice("obsw")

    def loadf(name, shape):
        t = consts.tile(shape, FP, name=f"c_{name}", uniquify=False)
        nc.sync.dma_start(t[:, :], dram[name][:, :])
        return t

    # split the d1 loads so early streams' layer-0 shortcut starts sooner
    d1r = consts.tile([128, 1024], FP, name="c_d1r", uniquify=False)
    d1i = consts.tile([128, 1024], FP, name="c_d1i", uniquify=False)
    for q in range(4):
        qs = slice(256 * q, 256 * q + 256)
        nc.sync.dma_start(d1r[:, qs], dram["d1r"][:, qs])
        nc.sync.dma_start(d1i[:, qs], dram["d1i"][:, qs])

    d2r = loadf("d2r", [128, 2048])
    d2i = loadf("d2i", [128, 2048])
    ones = consts.tile([128, 1], FP, name="ones", uniquify=False)
    nc.vector.memset(ones[:, :], 1.0)


    # stream views (4 independent batch quarters) ------------------------
    NS = 4          # streams
    CQ = 16 // NS   # c-chunks per stream (4)
    FQ = 128 * CQ   # free cols per stream tile (512)

    def bplanes(t):
        """B quarter-tile [128,512] -> (re, im) views [128, CQ, 64]."""
        v = t[:, :].rearrange("p (c r i) -> p r c i", r=2, i=64)
        return v[:, 0], v[:, 1]

    def pplanes(t):
        v = t[:, :].rearrange("p (r c i) -> p r c i", r=2, i=64)
        return v[:, 0], v[:, 1]

    def d1q(t2d, h):
        w = FQ // 2
        return t2d[:, w * h:w * h + w].rearrange("p (c i) -> p c i", i=64)

    def t3(t2d):
        return t2d[:, :].rearrange("p (c i) -> p c i", i=64)

    # ---------------- subroutines (per stream) --------------------------
    def diag_B(src_re, src_im, dst, h, from_psum=False):
        # GpSimd cannot read PSUM; when sources are PSUM all ops go to DVE
        eng2 = nc.vector if from_psum else nc.gpsimd
        dre, dim = bplanes(dst)
        t1 = temps.tile([128, FQ // 2], FP, name="t1", tag="t1")
        t2 = temps.tile([128, FQ // 2], FP, name="t2", tag="t2")
        t4 = temps.tile([128, FQ // 2], FP, name="t4", tag="t4")
        nc.vector.tensor_tensor(t3(t1), src_re, d1q(d1r, h), ALU.mult)
        eng2.tensor_tensor(t3(t2), src_im, d1q(d1i, h), ALU.mult)
        nc.vector.tensor_tensor(dre, t3(t1), t3(t2), ALU.subtract)
        nc.vector.tensor_tensor(t3(t1), src_re, d1q(d1i, h), ALU.mult)
        eng2.tensor_tensor(t3(t4), src_im, d1q(d1r, h), ALU.mult)
        nc.vector.tensor_tensor(dim, t3(t1), t3(t4), ALU.add)

    def jmult(trip, src, dst_psum):
        WrT, WiT, WmiT = trip
        sre, sim = bplanes(src)
        o_re = dst_psum[:, 0:FQ // 2]
        o_im = dst_psum[:, FQ // 2:FQ]
        nc.tensor.matmul(o_re, WrT, sre, start=True, stop=False)
        nc.tensor.matmul(o_re, WmiT, sim, start=False, stop=True)
        nc.tensor.matmul(o_im, WiT, sre, start=True, stop=False)
        nc.tensor.matmul(o_im, WrT, sim, start=False, stop=True)

    def jmult2(trip0, src0, trip1, src1, dst_psum):
        s0re, s0im = bplanes(src0)
        s1re, s1im = bplanes(src1)
        o_re = dst_psum[:, 0:FQ // 2]
        o_im = dst_psum[:, FQ // 2:FQ]
        nc.tensor.matmul(o_re, trip0[0], s0re, start=True, stop=False)
        nc.tensor.matmul(o_re, trip0[2], s0im, start=False, stop=False)
        nc.tensor.matmul(o_re, trip1[0], s1re, start=False, stop=False)
        nc.tensor.matmul(o_re, trip1[2], s1im, start=False, stop=True)
        nc.tensor.matmul(o_im, trip0[1], s0re, start=True, stop=False)
        nc.tensor.matmul(o_im, trip0[0], s0im, start=False, stop=False)
        nc.tensor.matmul(o_im, trip1[1], s1re, start=False, stop=False)
        nc.tensor.matmul(o_im, trip1[0], s1im, start=False, stop=True)

    def imult(lhsT, src, dst_psum):
        nc.tensor.matmul(dst_psum[:, :], lhsT, src[:, :], start=True, stop=True)

    def imult2(lhsT_a, src_a, lhsT_b, src_b, dst_psum):
        nc.tensor.matmul(dst_psum[:, :], lhsT_a, src_a[:, :], start=True, stop=False)
        nc.tensor.matmul(dst_psum[:, :], lhsT_b, src_b[:, :], start=False, stop=True)

    def copyback(dst, src_psum):
        nc.scalar.copy(dst[:, :], src_psum[:, :])

    def flip(src, dst):
        tp = tpsums.tile([128, FQ], FW, name="tp", tag="tp")
        for c in range(CQ):
            nc.tensor.transpose(tp[:, 128 * c:128 * c + 128],
                                src[:, 128 * c:128 * c + 128], ident)
        nc.scalar.copy(dst[:, :], tp[:, :])

    # ---------------- pipeline ------------------------------------------
    cur_planes = [None] * NS
    res = states.tile([1, 32], FP, name="res", tag="res", bufs=1)

    for l in range(4):
        SB2s = [None] * NS
        SB3s = [None] * NS
        SAs = [None] * NS
        pls = [None] * NS
        SA2s = [None] * NS
        SA3s = [None] * NS
        SA4s = [None] * NS
        SBas = [None] * NS
        SBbs = [None] * NS
        for h in range(NS):
            SB2s[h] = states.tile([128, FQ], FW, name=f"SB2_{h}", tag="SB2")
            if l == 0:
                # initial state is uniform 1/64 (real): Dz1 result = tables/64
                dre, dim = bplanes(SB2s[h])
                nc.scalar.mul(dre, d1q(d1r, h), 1.0 / 64.0)
                nc.scalar.mul(dim, d1q(d1i, h), 1.0 / 64.0)
            else:
                diag_B(cur_planes[h][0], cur_planes[h][1], SB2s[h], h,
                       from_psum=True)
        for h in range(NS):
            pk = psums.tile([128, FQ], FP, name=f"pk{h}", tag="pstate")
            jmult(k2r, SB2s[h], pk)
            SB3s[h] = states.tile([128, FQ], FW, name=f"SB3_{h}", tag="SB3")
            sb3re, sb3im = bplanes(SB3s[h])
            pkre, pkim = pplanes(pk)
            nc.scalar.copy(sb3re, pkre)
            nc.scalar.copy(sb3im, pkim)
        for h in range(NS):
            SAs[h] = states.tile([128, FQ], FW, name=f"SA{h}", tag="SA")
            flip(SB3s[h], SAs[h])
        for h in range(NS):
            pls[h] = psums.tile([128, FQ], FP, name=f"pl{h}", tag="pstate")
            imult(k2l, SAs[h], pls[h])
        for h in range(NS):
            pl = pls[h]
            hs = slice(FQ * h, FQ * h + FQ)
            SA2s[h] = states.tile([128, FQ], FW, name=f"SA2_{h}", tag="SA2")
            t1s = temps.tile([128, FQ], FP, name="t1a", tag="t1a")
            t2p = psums.tile([128, FQ], FP, name=f"t2p{h}", tag="t2p", bufs=2)
            nc.vector.tensor_tensor(t1s[:, :], pl[:, :], d2r[:, hs], ALU.mult)
            nc.vector.tensor_tensor(t2p[:, :], pl[:, :], d2i[:, hs], ALU.mult)
            nc.vector.tensor_tensor(SA2s[h][0:64, :], t1s[0:64, :], t2p[64:128, :],
                                    ALU.subtract)
            nc.vector.tensor_tensor(SA2s[h][64:128, :], t1s[64:128, :], t2p[0:64, :],
                                    ALU.add)

        if l == 3:
            for h in range(NS):
                po = psums.tile([128, FQ], FP, name=f"po{h}", tag="pstate")
                imult(obsw, SA2s[h], po)
                PR = states.tile([128, FQ], FP, name=f"PR{h}", tag="SB3")
                sa2f = SA2s[h][:, :].bitcast(FP) if use_f32r else SA2s[h][:, :]
                nc.vector.tensor_tensor(PR[:, :], sa2f, po[:, :], ALU.mult)
                ep = tpsums.tile([1, FQ], FP, name="ep", tag="tp")
                nc.tensor.matmul(ep[:, :], ones[:, :], PR[:, :],
                                 start=True, stop=True)
                epv = ep[:, :].rearrange("p (b j) -> p b j", j=64)
                ro = 8 * h
                nc.vector.tensor_reduce(res[:, ro:ro + 8], epv,
                                        axis=mybir.AxisListType.X, op=ALU.add)
        else:
            for h in range(NS):
                pu = psums.tile([128, FQ], FP, name=f"pu{h}", tag="pstate")
                SA3s[h] = states.tile([128, FQ], FW, name=f"SA3_{h}", tag="SA3")
                imult(u0[l], SA2s[h], pu)
                copyback(SA3s[h], pu)
                pu2 = psums.tile([128, FQ], FP, name=f"pu2{h}", tag="pstate")
                SA4s[h] = states.tile([128, FQ], FW, name=f"SA4_{h}", tag="SA4")
                imult(u1[l], SA2s[h], pu2)
                copyback(SA4s[h], pu2)
            for h in range(NS):
                SBas[h] = states.tile([128, FQ], FW, name=f"SBa{h}", tag="SB3")
                flip(SA3s[h], SBas[h])
                SBbs[h] = states.tile([128, FQ], FW, name=f"SBb{h}", tag="SA")
                flip(SA4s[h], SBbs[h])
            for h in range(NS):
                pv = psums.tile([128, FQ], FP, name=f"pv{h}", tag="pstate")
                jmult2(v0[l], SBas[h], v1[l], SBbs[h], pv)
                # next layer's Dz1 reads this PSUM directly (no copyback)
                cur_planes[h] = pplanes(pv)

    nc.sync.dma_start(dram["out"][:, :], res[:, :])


# ======================================================================
# public entry point
# ======================================================================
_CACHED = {}


def _build_program(use_f32r=True):
    """Build + compile the (input-independent) bass program once."""
    key = bool(use_f32r)
    if key in _CACHED:
        return _CACHED[key]
    from contextlib import ExitStack
    import concourse.bacc as bacc
    import concourse.mybir as mybir
    import concourse.tile as tile

    nc = bacc.Bacc("TRN2", target_bir_lowering=False, debug=False,
                   enable_asserts=True)
    _, packw1 = pack_offsets(PACK1_ORDER)
    _, packw2 = pack_offsets(PACK2_ORDER)
    shapes = {"cpack": [128, packw1], "upack": [128, packw2],
              "d1r": [128, 1024], "d1i": [128, 1024],
              "d2r": [128, 2048], "d2i": [128, 2048]}
    dtmap = dram_dtypes(use_f32r)
    dram = {}
    for name, shape in shapes.items():
        dram[name] = nc.dram_tensor(
            name, shape, dtmap.get(name, mybir.dt.float32),
            kind="ExternalInput").ap()
    dram["out"] = nc.dram_tensor("out", [1, 32], mybir.dt.float32,
                                 kind="ExternalOutput").ap()
    with tile.TileContext(nc) as tc:
        with ExitStack() as ctx:
            emit(ctx, tc, dram, use_f32r=use_f32r)
    nc.compile()
    _CACHED[key] = nc
    return nc


def kernel(x, params):
    """Full-input entry point: x (256,) f32, params (144,) f32 -> (256,) f32.

    Shards the batch over 8 NeuronCores (32 per core), runs the Bass/Tile
    statevector kernel SPMD, gathers per-core expectation values.
    """
    from concourse.bass_utils import run_bass_kernel_spmd

    x = np.asarray(x, dtype=np.float32).reshape(BATCH)
    params = np.asarray(params, dtype=np.float32).reshape(N_QUBITS * N_LAYERS * 3)
    nc = _build_program(use_f32r=True)
    in_maps = build_host_data(x, params)
    res = run_bass_kernel_spmd(nc, in_maps, list(range(N_CORES)))
    out = np.concatenate([res.results[c]["out"].reshape(BPC)
                          for c in range(N_CORES)])
    return out.astype(np.float32)
